# revision 1
# baseline (speedup 1.0000x reference)
"""Trainium2 Bass kernel for windowed (local) causal self-attention.

Reference computation (per batch element, fp32):
    q = x @ Wq.T + bq ; k = x @ Wk.T + bk ; v = x @ Wv.T + bv
    per non-overlapping window of 256 tokens:
        attn = softmax(causal_mask(q k^T * HEAD_DIM**-0.5))
        out  = attn @ v
    o = out @ Wo.T + bo + x

Sharding: data-parallel over (batch, window): 64 window-blocks of 256
tokens -> 8 cores x 8 windows.  Weights replicated.

Per-core kernel strategy:
  - all four transposed weights (W.T, [e_in, e_out]) resident in SBUF as
    float32r (PE matmul dtype: 1 cycle/row at N>=256, measured
    bit-identical to the PE's fp32 matmul, which runs 4 cycles/row).
  - x is transposed on the host and streamed per window as xT [E, 256]
    (kills 128 PE transposes + DVE evacs per core).
  - per window: qT/kT = Wm.T^T @ xT in [e_out, t] layout (bias fused into
    the ACT psum evacuation); scores accumulate over 8 K-tiles; causal
    mask added from an inline constant during psum evac; scale+exp+row-sum
    fused in one ACT op (accum_out); attn normalized by 1/sum on DVE,
    PE-transposed to attnT; v token-major; outT = v^T @ attnT (+bv in ACT
    evac); o = outT^T @ Wo.T (+bo via K=1 ones-matmul into the same psum
    group) + x residual fused into the DVE evacuation.
  - window-0 weight DMA (16MB) is the critical serial phase: loads are
    chunked and interleaved with compute emission, with tiny PE warmup
    transposes paced by arriving chunks to keep the HAM clock at 2.4GHz;
    output stores are deferred behind the next window's loads to avoid
    head-of-line blocking on the sync DMA queue.
"""
import sys

sys.path.insert(0, "/opt/trn_rl_repo")

import numpy as np

import concourse.bass as bass
import concourse.bacc as bacc
import concourse.mybir as mybir
import concourse.tile as tile
from concourse.bass_utils import run_bass_kernel_spmd

F32 = mybir.dt.float32
F32R = mybir.dt.float32r
AF = mybir.ActivationFunctionType

E = 1024          # embed dim
ET = E // 128     # e-tiles
W = 256           # window size
NW = 8            # windows per core
T = NW * W        # tokens per core
N_CORES = 8
SCALE = (E // 16) ** (-0.5)  # HEAD_DIM ** -0.5 = 0.125
NEG = -1.0e30


def build_nc(nw=NW):
    t_core = nw * W
    nc = bacc.Bacc("TRN2", target_bir_lowering=False, debug=False)

    x_d = nc.dram_tensor("x", [t_core, E], F32R, kind="ExternalInput")
    xt_d = nc.dram_tensor("xt", [E, t_core], F32R, kind="ExternalInput")
    w_d = {
        m: nc.dram_tensor(f"w{m}", [E, E], F32R, kind="ExternalInput")
        for m in ("q", "k", "v", "o")
    }
    bq_d = nc.dram_tensor("bq", [128, ET], F32, kind="ExternalInput")
    bk_d = nc.dram_tensor("bk", [128, ET], F32, kind="ExternalInput")
    bv_d = nc.dram_tensor("bv", [128, ET], F32, kind="ExternalInput")
    bo_d = nc.dram_tensor("bo", [1, E], F32R, kind="ExternalInput")
    o_d = nc.dram_tensor("o", [t_core, E], F32, kind="ExternalOutput")

    # host-side constants baked into the NEFF
    mask_np = np.zeros((2, 128, W), dtype=np.float32)
    for qt in range(2):
        r = np.arange(128)[:, None] + qt * 128
        c = np.arange(W)[None, :]
        mask_np[qt][c > r] = NEG
    mask_d = nc.inline_tensor(mask_np, "mask")
    ident_d = nc.inline_tensor(np.eye(128, dtype=np.float32), "ident")
    ones_d = nc.inline_tensor(np.ones((1, 128), dtype=np.float32), "ones")

    with tile.TileContext(nc) as tc:
        with (
            tc.tile_pool(name="wp", bufs=1) as wp,
            tc.tile_pool(name="cp", bufs=1) as cp,
            tc.tile_pool(name="xp", bufs=3) as xp,
            tc.tile_pool(name="xtp", bufs=2) as xtp,
            tc.tile_pool(name="qtp", bufs=1) as qtp,
            tc.tile_pool(name="ktp", bufs=1) as ktp,
            tc.tile_pool(name="otp", bufs=1) as otp,
            tc.tile_pool(name="vp", bufs=2) as vp,
            tc.tile_pool(name="sp", bufs=2) as sp,
            tc.tile_pool(name="ap_", bufs=2) as apool,
            tc.tile_pool(name="atp", bufs=2) as atp,
            tc.tile_pool(name="smp", bufs=4) as smp,
            tc.tile_pool(name="op", bufs=3) as op,
            tc.tile_pool(name="ps_qk", bufs=4, space=bass.MemorySpace.PSUM) as ps_qk,
            tc.tile_pool(name="ps_big", bufs=3, space=bass.MemorySpace.PSUM) as ps_big,
            tc.tile_pool(name="ps_tr", bufs=1, space=bass.MemorySpace.PSUM) as ps_tr,
        ):
            # ---- resident constants ----
            ident = cp.tile([128, 128], F32R, tag="ident")
            nc.gpsimd.dma_start(ident[:], ident_d.ap().bitcast(F32R))
            masks = cp.tile([128, 2, W], F32, tag="mask")
            for qt in range(2):
                nc.gpsimd.dma_start(masks[:, qt, :], mask_d.ap()[qt])
            ones = cp.tile([1, 128], F32R, tag="ones")
            nc.gpsimd.dma_start(ones[:], ones_d.ap().bitcast(F32R))
            bo_sb = cp.tile([1, E], F32R, tag="bo")
            nc.gpsimd.dma_start(bo_sb[:], bo_d.ap())
            bq_sb = cp.tile([128, ET], F32, tag="bq")
            nc.gpsimd.dma_start(bq_sb[:], bq_d.ap())
            bk_sb = cp.tile([128, ET], F32, tag="bk")
            nc.gpsimd.dma_start(bk_sb[:], bk_d.ap())
            bv_sb = cp.tile([128, ET], F32, tag="bv")
            nc.gpsimd.dma_start(bv_sb[:], bv_d.ap())

            # ---- resident weights: wsb[m][p, ei, eo] = W_m.T[ei*128+p, eo] ----
            # Weight DMAs are interleaved into window 0's emission below so
            # the sync engine starts x/window work immediately instead of
            # serializing 16MB of weight loads ahead of all compute.
            wsb = {}
            for m in ("q", "k", "v", "o"):
                wsb[m] = wp.tile([128, ET, E], F32R, tag=f"w{m}", name=f"w{m}sb")

            def load_weight(m, half=None, warm=False):
                # one 3D DMA per (half, ei-quadrant): 1MB transfers keep the
                # sync queue's ~0.6us/instr issue rate off the critical path
                wr = w_d[m].ap().rearrange("(a p) n -> a p n", p=128)
                halves = (0, 1) if half is None else (half,)
                for eoh in halves:
                    for eq in range(0, ET, 4):
                        nc.sync.dma_start(
                            wsb[m][:, eq : eq + 4, eoh * 512 : (eoh + 1) * 512],
                            wr[eq : eq + 4, :, eoh * 512 : (eoh + 1) * 512].transpose(
                                [1, 0, 2]
                            ),
                        )
                        if warm:
                            # keep the PE activity monitor warm through the
                            # DMA-bound phase: a tiny transpose per arriving
                            # chunk, paced by the DMA itself
                            wps = ps_tr.tile([128, 128], F32R, tag="tr", name="warm")
                            nc.tensor.transpose(
                                wps[:],
                                wsb[m][:, eq, eoh * 512 : eoh * 512 + 128],
                                ident[:],
                            )

            pending_stores = []

            def flush_stores():
                for dst, src_t in pending_stores:
                    nc.sync.dma_start(dst, src_t[:])
                pending_stores.clear()

            for w in range(nw):
                tok0 = w * W

                # ---- xT[p, ei, t] (e-major) loaded directly (host-transposed) ----
                xT = xtp.tile([128, ET, W], F32R, tag="xT")
                if w == 0:
                    # interleave xT and wq chunk loads so the first q-proj
                    # group starts as soon as the first chunk pair lands
                    wrq = w_d["q"].ap().rearrange("(a p) n -> a p n", p=128)
                    for ei in range(ET):
                        nc.sync.dma_start(
                            xT[:, ei, :],
                            xt_d.ap()[ei * 128 : (ei + 1) * 128, tok0 : tok0 + W],
                        )
                        nc.sync.dma_start(
                            wsb["q"][:, ei, 0:512], wrq[ei][:, 0:512]
                        )
                else:
                    xtr = xt_d.ap().rearrange("(a p) t -> a p t", p=128)
                    nc.sync.dma_start(
                        xT[:, :, :],
                        xtr[:, :, tok0 : tok0 + W].transpose([1, 0, 2]),
                    )
                # previous window's output stores go out behind our xT loads so
                # they never head-of-line-block the prefetch on the queue
                flush_stores()

                # ---- load x window (residual; not needed until o-proj) ----
                x_w = []
                if w > 0:
                    for tt in range(2):
                        xt_ = xp.tile([128, E], F32R, tag="x")
                        nc.sync.dma_start(
                            xt_[:], x_d.ap()[tok0 + tt * 128 : tok0 + (tt + 1) * 128, :]
                        )
                        x_w.append(xt_)

                if w == 0:
                    load_weight("q", half=1, warm=True)

                # ---- q/k projections -> [e_out, t] layout, bias fused ----
                qT = qtp.tile([128, ET, W], F32R, tag="qT")
                kT = ktp.tile([128, ET, W], F32R, tag="kT")
                for dst, m, b_sb in ((qT, "q", bq_sb), (kT, "k", bk_sb)):
                    if w == 0 and m == "k":
                        load_weight("k", half=1, warm=True)
                    for eo in range(ET):
                        if w == 0 and m == "q" and eo == 4:
                            load_weight("k", half=0, warm=True)
                        pp = ps_qk.tile([128, W], F32, tag="qk")
                        for ei in range(ET):
                            nc.tensor.matmul(
                                pp[:],
                                wsb[m][:, ei, eo * 128 : (eo + 1) * 128],
                                xT[:, ei, :],
                                start=(ei == 0),
                                stop=(ei == ET - 1),
                            )
                        nc.scalar.add(dst[:, eo, :], pp[:], b_sb[:, eo : eo + 1])

                if w == 0:
                    load_weight("v", half=0, warm=True)
                    for tt in range(2):
                        xt_ = xp.tile([128, E], F32R, tag="x", name="xt_w0")
                        nc.sync.dma_start(
                            xt_[:],
                            x_d.ap()[tok0 + tt * 128 : tok0 + (tt + 1) * 128, :],
                        )
                        x_w.append(xt_)

                # ---- scores + softmax + transpose(attn) ----
                aT = []
                for ktt in range(2):
                    t_ = atp.tile([128, W], F32R, tag="aT", name=f"aT{ktt}")
                    aT.append(t_)
                for qt in range(2):
                    sc = ps_qk.tile([128, W], F32, tag="qk")
                    for ei in range(ET):
                        nc.tensor.matmul(
                            sc[:],
                            qT[:, ei, qt * 128 : (qt + 1) * 128],
                            kT[:, ei, :],
                            start=(ei == 0),
                            stop=(ei == ET - 1),
                        )
                    s_sb = sp.tile([128, W], F32, tag="s")
                    nc.vector.tensor_add(s_sb[:], sc[:], masks[:, qt, :])
                    sums = smp.tile([128, 1], F32, tag="sum")
                    nc.scalar.activation(
                        s_sb[:], s_sb[:], AF.Exp, scale=SCALE, accum_out=sums[:]
                    )
                    rec = smp.tile([128, 1], F32, tag="rec")
                    nc.vector.reciprocal(rec[:], sums[:])
                    a_sb = apool.tile([128, W], F32R, tag="a")
                    nc.vector.tensor_scalar_mul(a_sb[:], s_sb[:], rec[:])
                    # transpose attn block rows->cols: aT[ktt][:, qt*128:...]
                    for ktt in range(2):
                        ptr = ps_tr.tile([128, 128], F32R, tag="tr", name="ptra")
                        nc.tensor.transpose(
                            ptr[:], a_sb[:, ktt * 128 : (ktt + 1) * 128], ident[:]
                        )
                        nc.vector.tensor_copy(
                            aT[ktt][:, qt * 128 : (qt + 1) * 128], ptr[:]
                        )

                if w == 0:
                    load_weight("o", half=0, warm=True)

                # ---- v projection (token-major) ----
                v_w = [vp.tile([128, E], F32R, tag="v", name=f"v{tt}") for tt in range(2)]
                for eoh in range(2):
                    if w == 0 and eoh == 1:
                        load_weight("v", half=1, warm=True)
                    for tt in range(2):
                        pv = ps_big.tile([128, 512], F32, tag="big")
                        for ei in range(ET):
                            nc.tensor.matmul(
                                pv[:],
                                xT[:, ei, tt * 128 : (tt + 1) * 128],
                                wsb["v"][:, ei, eoh * 512 : (eoh + 1) * 512],
                                start=(ei == 0),
                                stop=(ei == ET - 1),
                            )
                        nc.vector.tensor_copy(
                            v_w[tt][:, eoh * 512 : (eoh + 1) * 512], pv[:]
                        )

                # ---- attn @ v -> outT [e, t] layout, bias bv fused ----
                outT = otp.tile([128, ET, W], F32R, tag="outT")
                for et in range(ET):
                    pa = ps_qk.tile([128, W], F32, tag="qk")
                    for ktt in range(2):
                        nc.tensor.matmul(
                            pa[:],
                            v_w[ktt][:, et * 128 : (et + 1) * 128],
                            aT[ktt][:],
                            start=(ktt == 0),
                            stop=(ktt == 1),
                        )
                    nc.scalar.add(outT[:, et, :], pa[:], bv_sb[:, et : et + 1])

                # ---- output projection + bo + residual ----
                for eoh in range(2):
                    if w == 0 and eoh == 1:
                        load_weight("o", half=1, warm=True)
                    for tt in range(2):
                        po = ps_big.tile([128, 512], F32, tag="big")
                        for ei in range(ET):
                            nc.tensor.matmul(
                                po[:],
                                outT[:, ei, tt * 128 : (tt + 1) * 128],
                                wsb["o"][:, ei, eoh * 512 : (eoh + 1) * 512],
                                start=(ei == 0),
                                stop=False,
                            )
                        nc.tensor.matmul(
                            po[:],
                            ones[:],
                            bo_sb[:, eoh * 512 : (eoh + 1) * 512],
                            start=False,
                            stop=True,
                        )
                        o_sb = op.tile([128, 512], F32, tag="o")
                        nc.vector.tensor_add(
                            o_sb[:],
                            po[:],
                            x_w[tt][:, eoh * 512 : (eoh + 1) * 512].bitcast(F32),
                        )
                        dst_ap = o_d.ap()[
                            tok0 + tt * 128 : tok0 + (tt + 1) * 128,
                            eoh * 512 : (eoh + 1) * 512,
                        ]
                        if w == nw - 1:
                            nc.sync.dma_start(dst_ap, o_sb[:])
                        else:
                            pending_stores.append((dst_ap, o_sb))

            flush_stores()

    nc.compile()
    return nc


_NC_CACHE = {}


def _get_nc(nw=NW):
    if nw not in _NC_CACHE:
        _NC_CACHE[nw] = build_nc(nw)
    return _NC_CACHE[nw]


def kernel(x, Wq, bq, Wk, bk, Wv, bv, Wo, bo):
    x = np.asarray(x, dtype=np.float32)
    B, S, _ = x.shape
    x_flat = np.ascontiguousarray(x.reshape(B * S, E))
    t_core = B * S // N_CORES
    assert t_core == T

    common = {
        "wq": np.ascontiguousarray(np.asarray(Wq, np.float32).T),
        "wk": np.ascontiguousarray(np.asarray(Wk, np.float32).T),
        "wv": np.ascontiguousarray(np.asarray(Wv, np.float32).T),
        "wo": np.ascontiguousarray(np.asarray(Wo, np.float32).T),
        "bq": np.ascontiguousarray(np.asarray(bq, np.float32).reshape(ET, 128).T),
        "bk": np.ascontiguousarray(np.asarray(bk, np.float32).reshape(ET, 128).T),
        "bv": np.ascontiguousarray(np.asarray(bv, np.float32).reshape(ET, 128).T),
        "bo": np.ascontiguousarray(np.asarray(bo, np.float32).reshape(1, E)),
    }
    in_maps = [
        {
            "x": np.ascontiguousarray(x_flat[i * t_core : (i + 1) * t_core]),
            "xt": np.ascontiguousarray(x_flat[i * t_core : (i + 1) * t_core].T),
            **common,
        }
        for i in range(N_CORES)
    ]

    nc = _get_nc()
    res = run_bass_kernel_spmd(nc, in_maps, core_ids=list(range(N_CORES)))
    out = np.concatenate([res.results[i]["o"] for i in range(N_CORES)], axis=0)
    return out.reshape(B, S, E).astype(np.float32)



# revision 12
# speedup vs baseline: 1.7803x; 1.7803x over previous
"""Trainium2 Bass kernel for windowed (local) causal self-attention.

Reference computation (per batch element, fp32):
    q = x @ Wq.T + bq ; k = x @ Wk.T + bk ; v = x @ Wv.T + bv
    per non-overlapping window of 256 tokens:
        attn = softmax(causal_mask(q k^T * HEAD_DIM**-0.5))
        out  = attn @ v
    o = out @ Wo.T + bo + x

Algebraic restructure (no head split in this module, softmax rows sum to 1):
    scores = q k^T = x M x^T + cq 1^T + 1 ck^T + bq.bk,  M  = Wq^T Wk
        cq = x (Wq^T bk),  ck = x (Wk^T bq)   (host-computed per-token rows)
    o = attn (x N) + (bv Wo^T + bo) + x,      N  = Wv^T Wo^T
so only TWO E x E projections remain on device (q' = x M and v' = x N);
M, N, cq, ck, bq.bk and the constant output row are computed on the host
in float64.  The residual + constant row are also added on the host.

Sharding: data-parallel over (batch, window): 64 window-blocks of 256
tokens -> 8 cores x 8 windows.  M, N replicated.

Per-core kernel strategy (all fp32r on the PE):
  - scores are computed TRANSPOSED, sT[k, q] = x_k . q'_q, so no PE
    transposes of the attention matrix are needed: exp(sT) chunks serve
    directly as the stationary operand of out = attn @ v'.
  - softmax row sums become N=1 matmuls (expT^T @ ones) accumulated over
    k-chunks; normalization is folded into the ACT output evacuation as a
    per-partition scale (1/sum).
  - causal mask + bq.bk live in an inline constant added on DVE; the
    cq/ck bias rows enter the score psum via one K=2 matmul per k-chunk.
  - v' is computed token-major between the score matmuls and the
    attention matmuls so the PE stays busy through the softmax chain.
  - window-0 M/N DMA is chunked and interleaved with compute emission,
    with tiny PE warmup transposes paced by arriving chunks; xT loads are
    prefetched one window ahead; output stores are deferred behind the
    next window's loads.
"""
import sys

sys.path.insert(0, "/opt/trn_rl_repo")

import numpy as np

import concourse.bass as bass
import concourse.bacc as bacc
import concourse.mybir as mybir
import concourse.tile as tile
from concourse.bass_utils import run_bass_kernel_spmd

F32 = mybir.dt.float32
F32R = mybir.dt.float32r
AF = mybir.ActivationFunctionType

E = 1024          # embed dim
ET = E // 128     # e-tiles
W = 256           # window size
NW = 8            # windows per core
T = NW * W        # tokens per core
N_CORES = 8
SCALE = (E // 16) ** (-0.5)  # HEAD_DIM ** -0.5 = 0.125
NEG = -1.0e30


def build_nc(nw=NW):
    t_core = nw * W
    nc = bacc.Bacc("TRN2", target_bir_lowering=False, debug=False)

    xt_d = nc.dram_tensor("xt", [E, t_core], F32R, kind="ExternalInput")
    m_d = nc.dram_tensor("m", [E, E], F32R, kind="ExternalInput")
    n_d = nc.dram_tensor("n", [E, E], F32R, kind="ExternalInput")
    kql_d = nc.dram_tensor("kql", [2, t_core], F32R, kind="ExternalInput")
    kqr_d = nc.dram_tensor("kqr", [2, t_core], F32R, kind="ExternalInput")
    o_d = nc.dram_tensor("o", [t_core, E], F32, kind="ExternalOutput")

    # transposed causal mask constant: maskT[kt][kp, q] applies to
    # sT[k, q] = score(q, k).  (The bq.bk score constant is dropped: a
    # uniform shift of every visible logit cancels in softmax.)
    mask_np = np.full((2, 128, W), NEG, dtype=np.float32)
    for kt in range(2):
        k = np.arange(128)[:, None] + kt * 128
        q = np.arange(W)[None, :]
        mask_np[kt][k <= q] = 0.0
    mask_d = nc.inline_tensor(mask_np, "mask")
    ident_d = nc.inline_tensor(np.eye(128, dtype=np.float32), "ident")
    # two identical ones-columns: fp32r matmul dst free size must be even,
    # so the softmax row sums are computed as N=2 (duplicate) columns
    onec_d = nc.inline_tensor(np.ones((128, 2), dtype=np.float32), "onec")

    with tile.TileContext(nc) as tc:
        with (
            tc.tile_pool(name="wp", bufs=1) as wp,
            tc.tile_pool(name="cp", bufs=1) as cp,
            tc.tile_pool(name="xtp", bufs=3) as xtp,
            tc.tile_pool(name="qtp", bufs=2) as qtp,
            tc.tile_pool(name="etp", bufs=2) as etp,
            tc.tile_pool(name="sp", bufs=4) as sp,
            tc.tile_pool(name="vp", bufs=4) as vp,
            tc.tile_pool(name="smp", bufs=8) as smp,
            tc.tile_pool(name="op", bufs=4) as op,
            tc.tile_pool(name="ps_qk", bufs=3, space=bass.MemorySpace.PSUM) as ps_qk,
            tc.tile_pool(name="ps_big", bufs=3, space=bass.MemorySpace.PSUM) as ps_big,
            tc.tile_pool(name="ps_sm", bufs=1, space=bass.MemorySpace.PSUM) as ps_sm,
            tc.tile_pool(name="ps_wm", bufs=1, space=bass.MemorySpace.PSUM) as ps_wm,
        ):
            # ---- resident constants ----
            ident = cp.tile([128, 128], F32R, tag="ident")
            nc.gpsimd.dma_start(ident[:], ident_d.ap().bitcast(F32R))
            masks = cp.tile([128, 2, W], F32, tag="mask")
            for kt in range(2):
                nc.gpsimd.dma_start(masks[:, kt, :], mask_d.ap()[kt])
            onec = cp.tile([128, 2], F32R, tag="onec")
            nc.gpsimd.dma_start(onec[:], onec_d.ap().bitcast(F32R))
            kql = cp.tile([2, t_core], F32R, tag="kql")
            nc.gpsimd.dma_start(kql[:], kql_d.ap())
            kqr = cp.tile([2, t_core], F32R, tag="kqr")
            nc.gpsimd.dma_start(kqr[:], kqr_d.ap())

            # ---- resident weights: [p, ei, eo] = Wmat[ei*128+p, eo] ----
            msb = wp.tile([128, ET, E], F32R, tag="m", name="msb")
            nsb = wp.tile([128, ET, E], F32R, tag="n", name="nsb")
            m_r = m_d.ap().rearrange("(a p) n -> a p n", p=128)
            n_r = n_d.ap().rearrange("(a p) n -> a p n", p=128)

            def warm():
                # keep the PE activity monitor warm through the DMA-bound
                # phase: a tiny transpose per arriving chunk, paced by the
                # DMA itself
                wps = ps_wm.tile([128, 128], F32R, tag="warm", name="warm")
                nc.tensor.transpose(wps[:], msb[:, 0, 0:128], ident[:])

            pending_stores = []

            def flush_stores():
                for dst, src_t in pending_stores:
                    nc.sync.dma_start(dst, src_t[:])
                pending_stores.clear()

            xT_next = None
            for w in range(nw):
                tok0 = w * W

                # ---- xT[p, ei, t] (e-major, host-transposed) ----
                if w == 0:
                    xT = xtp.tile([128, ET, W], F32R, tag="xT")
                    # interleave M column-chunks with the w0 xT load so the
                    # first q'-proj group starts as soon as chunk 0 lands
                    nc.sync.dma_start(
                        msb[:, :, 0:128], m_r[:, :, 0:128].transpose([1, 0, 2])
                    )
                    warm()
                    xtr = xt_d.ap().rearrange("(a p) t -> a p t", p=128)
                    nc.sync.dma_start(
                        xT[:, :, :], xtr[:, :, tok0 : tok0 + W].transpose([1, 0, 2])
                    )
                    for eo in range(1, ET):
                        nc.sync.dma_start(
                            msb[:, :, eo * 128 : (eo + 1) * 128],
                            m_r[:, :, eo * 128 : (eo + 1) * 128].transpose([1, 0, 2]),
                        )
                        warm()
                else:
                    xT = xT_next

                # ---- q' projection -> q'T [e_out, t] ----
                qT = qtp.tile([128, ET, W], F32R, tag="qT")
                for eo in range(ET):
                    pp = ps_qk.tile([128, W], F32, tag="qk")
                    for ei in range(ET):
                        nc.tensor.matmul(
                            pp[:],
                            msb[:, ei, eo * 128 : (eo + 1) * 128],
                            xT[:, ei, :],
                            start=(ei == 0),
                            stop=(ei == ET - 1),
                        )
                    nc.scalar.copy(qT[:, eo, :], pp[:])

                # prefetch next window's xT behind this window's compute
                if w + 1 < nw:
                    xT_next = xtp.tile([128, ET, W], F32R, tag="xT")
                    xtr = xt_d.ap().rearrange("(a p) t -> a p t", p=128)
                    nc.sync.dma_start(
                        xT_next[:, :, :],
                        xtr[:, :, tok0 + W : tok0 + 2 * W].transpose([1, 0, 2]),
                    )
                # previous window's output stores go out behind the prefetch
                flush_stores()

                # ---- transposed scores sT[k, q] + softmax ----
                expT = etp.tile([128, 2, W], F32R, tag="expT")
                for kt in range(2):
                    sc = ps_qk.tile([128, W], F32, tag="qk")
                    # cq/ck bias rows: sT[k, q] += ck[k] + cq[q]
                    nc.tensor.matmul(
                        sc[:],
                        kql[:, tok0 + kt * 128 : tok0 + (kt + 1) * 128],
                        kqr[:, tok0 : tok0 + W],
                        start=True,
                        stop=False,
                    )
                    for ei in range(ET):
                        nc.tensor.matmul(
                            sc[:],
                            xT[:, ei, kt * 128 : (kt + 1) * 128],
                            qT[:, ei, :],
                            start=False,
                            stop=(ei == ET - 1),
                        )
                    s_sb = sp.tile([128, W], F32, tag="s")
                    nc.vector.tensor_add(s_sb[:], sc[:], masks[:, kt, :])
                    nc.scalar.activation(expT[:, kt, :], s_sb[:], AF.Exp, scale=SCALE)

                if w == 0:
                    # N chunk loads slot in behind the w0 score matmuls
                    for half in range(2):
                        for eq in range(0, ET, 4):
                            nc.sync.dma_start(
                                nsb[:, eq : eq + 4, half * 512 : (half + 1) * 512],
                                n_r[
                                    eq : eq + 4, :, half * 512 : (half + 1) * 512
                                ].transpose([1, 0, 2]),
                            )
                            warm()

                # ---- v' projection (token-major), fills PE during softmax ----
                v_w = [vp.tile([128, E], F32R, tag="v", name=f"v{kt}") for kt in range(2)]
                for kt in range(2):
                    for eoh in range(2):
                        pv = ps_big.tile([128, 512], F32, tag="big")
                        for ei in range(ET):
                            nc.tensor.matmul(
                                pv[:],
                                xT[:, ei, kt * 128 : (kt + 1) * 128],
                                nsb[:, ei, eoh * 512 : (eoh + 1) * 512],
                                start=(ei == 0),
                                stop=(ei == ET - 1),
                            )
                        nc.vector.tensor_copy(
                            v_w[kt][:, eoh * 512 : (eoh + 1) * 512], pv[:]
                        )

                # ---- softmax row sums (over k = partitions) via N=1 matmuls ----
                recs = []
                for qt in range(2):
                    sm = ps_sm.tile([128, 2], F32, tag="sum")
                    for kt in range(2):
                        nc.tensor.matmul(
                            sm[:],
                            expT[:, kt, qt * 128 : (qt + 1) * 128],
                            onec[:],
                            start=(kt == 0),
                            stop=(kt == 1),
                        )
                    rec = smp.tile([128, 1], F32, tag="rec")
                    nc.vector.reciprocal(rec[:], sm[:, 0:1])
                    recs.append(rec)

                # ---- out = attn @ v' (token-major), normalize in ACT evac ----
                for qt in range(2):
                    o_sb = op.tile([128, E], F32, tag="o")
                    for eoh in range(2):
                        po = ps_big.tile([128, 512], F32, tag="big")
                        for kt in range(2):
                            nc.tensor.matmul(
                                po[:],
                                expT[:, kt, qt * 128 : (qt + 1) * 128],
                                v_w[kt][:, eoh * 512 : (eoh + 1) * 512],
                                start=(kt == 0),
                                stop=(kt == 1),
                            )
                        nc.scalar.activation(
                            o_sb[:, eoh * 512 : (eoh + 1) * 512],
                            po[:],
                            AF.Copy,
                            scale=recs[qt][:],
                        )
                    dst_ap = o_d.ap()[tok0 + qt * 128 : tok0 + (qt + 1) * 128, :]
                    if w == nw - 1:
                        nc.sync.dma_start(dst_ap, o_sb[:])
                    else:
                        pending_stores.append((dst_ap, o_sb))

            flush_stores()

    nc.compile()
    return nc


_NC_CACHE = {}


def _get_nc(nw=NW):
    if nw not in _NC_CACHE:
        _NC_CACHE[nw] = build_nc(nw)
    return _NC_CACHE[nw]


def prepare(x, Wq, bq, Wk, bk, Wv, bv, Wo, bo):
    """Host-side precompute: per-core input maps + host residual terms."""
    x = np.asarray(x, dtype=np.float32)
    B, S, _ = x.shape
    x_flat = np.ascontiguousarray(x.reshape(B * S, E))
    t_core = B * S // N_CORES
    assert t_core == T

    f64 = np.float64
    Wq64, Wk64 = np.asarray(Wq, f64), np.asarray(Wk, f64)
    Wv64, Wo64 = np.asarray(Wv, f64), np.asarray(Wo, f64)
    bq64, bk64 = np.asarray(bq, f64), np.asarray(bk, f64)
    bv64, bo64 = np.asarray(bv, f64), np.asarray(bo, f64)

    M = np.ascontiguousarray((Wq64.T @ Wk64).astype(np.float32))
    N = np.ascontiguousarray((Wv64.T @ Wo64.T).astype(np.float32))
    cq = (x_flat.astype(f64) @ (Wq64.T @ bk64)).astype(np.float32)  # [T_total]
    ck = (x_flat.astype(f64) @ (Wk64.T @ bq64)).astype(np.float32)
    orow = (bv64 @ Wo64.T + bo64).astype(np.float32)  # [E]

    ones_t = np.ones(B * S, dtype=np.float32)
    kql_full = np.ascontiguousarray(np.stack([ck, ones_t]))  # [2, T_total]
    kqr_full = np.ascontiguousarray(np.stack([ones_t, cq]))

    common = {"m": M, "n": N}
    in_maps = [
        {
            "xt": np.ascontiguousarray(x_flat[i * t_core : (i + 1) * t_core].T),
            "kql": np.ascontiguousarray(kql_full[:, i * t_core : (i + 1) * t_core]),
            "kqr": np.ascontiguousarray(kqr_full[:, i * t_core : (i + 1) * t_core]),
            **common,
        }
        for i in range(N_CORES)
    ]
    return in_maps, orow, x_flat, (B, S)


def kernel(x, Wq, bq, Wk, bk, Wv, bv, Wo, bo):
    in_maps, orow, x_flat, (B, S) = prepare(x, Wq, bq, Wk, bk, Wv, bv, Wo, bo)
    nc = _get_nc()
    res = run_bass_kernel_spmd(nc, in_maps, core_ids=list(range(N_CORES)))
    out = np.concatenate([res.results[i]["o"] for i in range(N_CORES)], axis=0)
    out += orow[None, :]
    out += x_flat
    return out.reshape(B, S, E).astype(np.float32)


# revision 13
# speedup vs baseline: 1.8690x; 1.0498x over previous
"""Trainium2 Bass kernel for windowed (local) causal self-attention.

Reference computation (per batch element, fp32):
    q = x @ Wq.T + bq ; k = x @ Wk.T + bk ; v = x @ Wv.T + bv
    per non-overlapping window of 256 tokens:
        attn = softmax(causal_mask(q k^T * HEAD_DIM**-0.5))
        out  = attn @ v
    o = out @ Wo.T + bo + x

Algebraic restructure (no head split in this module, softmax rows sum to 1):
    scores = q k^T = x M x^T + cq 1^T + 1 ck^T + bq.bk,  M  = Wq^T Wk
        cq = x (Wq^T bk)  [per-QUERY shift: cancels in softmax, dropped]
        ck = x (Wk^T bq)  [per-KEY: folded into the ACT exp bias]
    o = attn (x N) + (bv Wo^T + bo) + x,      N  = Wv^T Wo^T
so only TWO E x E projections remain on device (q' = x M and v' = x N);
M, N, ck and the constant output row are computed on the host in float64.
The residual + constant row are also added on the host.

Sharding: data-parallel over (batch, window): 64 window-blocks of 256
tokens -> 8 cores x 8 windows.  M, N replicated.

Per-core kernel strategy (all fp32r on the PE):
  - scores are computed TRANSPOSED, sT[k, q] = x_k . q'_q, so no PE
    transposes of the attention matrix are needed: exp(sT) chunks serve
    directly as the stationary operand of out = attn @ v'.
  - softmax row sums become N=2 matmuls (expT^T @ ones2) accumulated over
    k-chunks; normalization is folded into the ACT output evacuation as a
    per-partition scale (1/sum).
  - q'-projection is window-PAIRED (moving 512 tokens) to halve its
    instruction count and PE weight-load switches.
  - v' is computed token-major between the score matmuls and the
    attention matmuls so the PE stays busy through the softmax chain.
  - window-0 M/N DMA is chunked and interleaved with compute emission,
    with tiny PE warmup transposes paced by arriving chunks; xT loads are
    prefetched one pair ahead; output stores are deferred behind the
    next pair's loads.
"""
import sys

sys.path.insert(0, "/opt/trn_rl_repo")

import numpy as np

import concourse.bass as bass
import concourse.bacc as bacc
import concourse.mybir as mybir
import concourse.tile as tile
from concourse.bass_utils import run_bass_kernel_spmd

F32 = mybir.dt.float32
F32R = mybir.dt.float32r
AF = mybir.ActivationFunctionType

E = 1024          # embed dim
ET = E // 128     # e-tiles
W = 256           # window size
NW = 8            # windows per core
T = NW * W        # tokens per core
N_CORES = 8
SCALE = (E // 16) ** (-0.5)  # HEAD_DIM ** -0.5 = 0.125
NEG = -1.0e30
PW = 2 * W        # tokens per window pair


def build_nc(nw=NW):
    t_core = nw * W
    npair = nw // 2
    nc = bacc.Bacc("TRN2", target_bir_lowering=False, debug=False)

    xt_d = nc.dram_tensor("xt", [E, t_core], F32R, kind="ExternalInput")
    m_d = nc.dram_tensor("m", [E, E], F32R, kind="ExternalInput")
    n_d = nc.dram_tensor("n", [E, E], F32R, kind="ExternalInput")
    # ck * SCALE laid out as one [128] column per 128-token chunk
    ckc_d = nc.dram_tensor("ckc", [128, 2 * nw], F32, kind="ExternalInput")
    o_d = nc.dram_tensor("o", [t_core, E], F32, kind="ExternalOutput")

    # transposed causal mask constant: maskT[kt][kp, q] applies to
    # sT[k, q] = score(q, k).  (The bq.bk score constant and the per-query
    # cq row are dropped: uniform per-row logit shifts cancel in softmax.)
    mask_np = np.full((2, 128, W), NEG, dtype=np.float32)
    for kt in range(2):
        k = np.arange(128)[:, None] + kt * 128
        q = np.arange(W)[None, :]
        mask_np[kt][k <= q] = 0.0
    mask_d = nc.inline_tensor(mask_np, "mask")
    ident_d = nc.inline_tensor(np.eye(128, dtype=np.float32), "ident")
    # two identical ones-columns: fp32r matmul dst free size must be even,
    # so the softmax row sums are computed as N=2 (duplicate) columns
    onec_d = nc.inline_tensor(np.ones((128, 2), dtype=np.float32), "onec")

    with tile.TileContext(nc) as tc:
        with (
            tc.tile_pool(name="wp", bufs=1) as wp,
            tc.tile_pool(name="cp", bufs=1) as cp,
            tc.tile_pool(name="xtp", bufs=2) as xtp,
            tc.tile_pool(name="qtp", bufs=2) as qtp,
            tc.tile_pool(name="etp", bufs=2) as etp,
            tc.tile_pool(name="sp", bufs=4) as sp,
            tc.tile_pool(name="vp", bufs=4) as vp,
            tc.tile_pool(name="smp", bufs=8) as smp,
            tc.tile_pool(name="op", bufs=4) as op,
            tc.tile_pool(name="ps_qk", bufs=3, space=bass.MemorySpace.PSUM) as ps_qk,
            tc.tile_pool(name="ps_big", bufs=3, space=bass.MemorySpace.PSUM) as ps_big,
            tc.tile_pool(name="ps_sm", bufs=1, space=bass.MemorySpace.PSUM) as ps_sm,
            tc.tile_pool(name="ps_wm", bufs=1, space=bass.MemorySpace.PSUM) as ps_wm,
        ):
            # ---- resident constants ----
            ident = cp.tile([128, 128], F32R, tag="ident")
            nc.gpsimd.dma_start(ident[:], ident_d.ap().bitcast(F32R))
            masks = cp.tile([128, 2, W], F32, tag="mask")
            for kt in range(2):
                nc.gpsimd.dma_start(masks[:, kt, :], mask_d.ap()[kt])
            onec = cp.tile([128, 2], F32R, tag="onec")
            nc.gpsimd.dma_start(onec[:], onec_d.ap().bitcast(F32R))
            ckc = cp.tile([128, 2 * nw], F32, tag="ckc")
            nc.gpsimd.dma_start(ckc[:], ckc_d.ap())

            # ---- resident weights: [p, ei, eo] = Wmat[ei*128+p, eo] ----
            msb = wp.tile([128, ET, E], F32R, tag="m", name="msb")
            nsb = wp.tile([128, ET, E], F32R, tag="n", name="nsb")
            m_r = m_d.ap().rearrange("(a p) n -> a p n", p=128)
            n_r = n_d.ap().rearrange("(a p) n -> a p n", p=128)

            def warm():
                # keep the PE activity monitor warm through the DMA-bound
                # phase: a tiny transpose per arriving chunk, paced by the
                # DMA itself
                wps = ps_wm.tile([128, 128], F32R, tag="warm", name="warm")
                nc.tensor.transpose(wps[:], msb[:, 0, 0:128], ident[:])

            pending_stores = []

            def flush_stores():
                for dst, src_t, sl in pending_stores:
                    nc.sync.dma_start(dst, src_t[sl[0] : sl[1], sl[2] : sl[3]])
                pending_stores.clear()

            xtr = xt_d.ap().rearrange("(a p) t -> a p t", p=128)
            xT_next = None
            for p in range(npair):
                ptok0 = p * PW

                # ---- xT[p, ei, t] for the pair (e-major, host-transposed) ----
                if p == 0:
                    xT = xtp.tile([128, ET, PW], F32R, tag="xT")
                    # interleave M column-chunks with the pair-0 xT load so
                    # the first q'-proj group starts as soon as chunk 0 lands
                    nc.sync.dma_start(
                        msb[:, :, 0:128], m_r[:, :, 0:128].transpose([1, 0, 2])
                    )
                    warm()
                    nc.sync.dma_start(
                        xT[:, :, :], xtr[:, :, 0:PW].transpose([1, 0, 2])
                    )
                    for eo in range(1, ET):
                        nc.sync.dma_start(
                            msb[:, :, eo * 128 : (eo + 1) * 128],
                            m_r[:, :, eo * 128 : (eo + 1) * 128].transpose([1, 0, 2]),
                        )
                        warm()
                else:
                    xT = xT_next

                # ---- q' projection for the pair -> q'T [e_out, t(512)] ----
                qT = qtp.tile([128, ET, PW], F32R, tag="qT")
                for eo in range(ET):
                    pp = ps_big.tile([128, PW], F32, tag="big")
                    for ei in range(ET):
                        nc.tensor.matmul(
                            pp[:],
                            msb[:, ei, eo * 128 : (eo + 1) * 128],
                            xT[:, ei, :],
                            start=(ei == 0),
                            stop=(ei == ET - 1),
                        )
                    nc.scalar.copy(qT[:, eo, :], pp[:])

                if p == 0:
                    # N chunk loads slot in behind the pair-0 q' matmuls
                    for half in range(2):
                        for eq in range(0, ET, 4):
                            nc.sync.dma_start(
                                nsb[:, eq : eq + 4, half * 512 : (half + 1) * 512],
                                n_r[
                                    eq : eq + 4, :, half * 512 : (half + 1) * 512
                                ].transpose([1, 0, 2]),
                            )
                            warm()

                # prefetch next pair's xT behind this pair's compute
                if p + 1 < npair:
                    xT_next = xtp.tile([128, ET, PW], F32R, tag="xT")
                    nc.sync.dma_start(
                        xT_next[:, :, :],
                        xtr[:, :, ptok0 + PW : ptok0 + 2 * PW].transpose([1, 0, 2]),
                    )
                # previous pair's output stores go out behind the prefetch
                flush_stores()

                for wi in range(2):
                    w = 2 * p + wi
                    tok0 = w * W
                    wt0 = wi * W  # token offset inside the pair tiles

                    # ---- transposed scores sT[k, q] + softmax ----
                    expT = etp.tile([128, 2, W], F32R, tag="expT")
                    for kt in range(2):
                        sc = ps_qk.tile([128, W], F32, tag="qk")
                        for ei in range(ET):
                            nc.tensor.matmul(
                                sc[:],
                                xT[:, ei, wt0 + kt * 128 : wt0 + (kt + 1) * 128],
                                qT[:, ei, wt0 : wt0 + W],
                                start=(ei == 0),
                                stop=(ei == ET - 1),
                            )
                        s_sb = sp.tile([128, W], F32, tag="s")
                        nc.vector.tensor_add(s_sb[:], sc[:], masks[:, kt, :])
                        # exp(SCALE*s + SCALE*ck[k]): per-key bias via ACT
                        nc.scalar.activation(
                            expT[:, kt, :],
                            s_sb[:],
                            AF.Exp,
                            scale=SCALE,
                            bias=ckc[:, 2 * w + kt : 2 * w + kt + 1],
                        )

                    # ---- v' projection (token-major), fills PE during softmax ----
                    v_w = [
                        vp.tile([128, E], F32R, tag="v", name=f"v{kt}")
                        for kt in range(2)
                    ]
                    for kt in range(2):
                        for eoh in range(2):
                            pv = ps_big.tile([128, 512], F32, tag="big")
                            for ei in range(ET):
                                nc.tensor.matmul(
                                    pv[:],
                                    xT[:, ei, wt0 + kt * 128 : wt0 + (kt + 1) * 128],
                                    nsb[:, ei, eoh * 512 : (eoh + 1) * 512],
                                    start=(ei == 0),
                                    stop=(ei == ET - 1),
                                )
                            nc.vector.tensor_copy(
                                v_w[kt][:, eoh * 512 : (eoh + 1) * 512], pv[:]
                            )

                    # ---- softmax row sums (over k = partitions) ----
                    recs = []
                    for qt in range(2):
                        sm = ps_sm.tile([128, 2], F32, tag="sum")
                        for kt in range(2):
                            nc.tensor.matmul(
                                sm[:],
                                expT[:, kt, qt * 128 : (qt + 1) * 128],
                                onec[:],
                                start=(kt == 0),
                                stop=(kt == 1),
                            )
                        rec = smp.tile([128, 1], F32, tag="rec")
                        nc.vector.reciprocal(rec[:], sm[:, 0:1])
                        recs.append(rec)

                    # ---- out = attn @ v' (token-major), normalize in evac ----
                    for qt in range(2):
                        o_sb = op.tile([128, E], F32, tag="o")
                        for eoh in range(2):
                            po = ps_big.tile([128, 512], F32, tag="big")
                            for kt in range(2):
                                nc.tensor.matmul(
                                    po[:],
                                    expT[:, kt, qt * 128 : (qt + 1) * 128],
                                    v_w[kt][:, eoh * 512 : (eoh + 1) * 512],
                                    start=(kt == 0),
                                    stop=(kt == 1),
                                )
                            nc.scalar.activation(
                                o_sb[:, eoh * 512 : (eoh + 1) * 512],
                                po[:],
                                AF.Copy,
                                scale=recs[qt][:],
                            )
                            if w == nw - 1:
                                # final window: stream each half out as soon
                                # as it is evacuated to shorten the tail
                                nc.sync.dma_start(
                                    o_d.ap()[
                                        tok0 + qt * 128 : tok0 + (qt + 1) * 128,
                                        eoh * 512 : (eoh + 1) * 512,
                                    ],
                                    o_sb[:, eoh * 512 : (eoh + 1) * 512],
                                )
                        if w < nw - 1:
                            dst_ap = o_d.ap()[
                                tok0 + qt * 128 : tok0 + (qt + 1) * 128, :
                            ]
                            pending_stores.append((dst_ap, o_sb, (0, 128, 0, E)))

            flush_stores()

    nc.compile()
    return nc


_NC_CACHE = {}


def _get_nc(nw=NW):
    if nw not in _NC_CACHE:
        _NC_CACHE[nw] = build_nc(nw)
    return _NC_CACHE[nw]


def prepare(x, Wq, bq, Wk, bk, Wv, bv, Wo, bo):
    """Host-side precompute: per-core input maps + host residual terms."""
    x = np.asarray(x, dtype=np.float32)
    B, S, _ = x.shape
    x_flat = np.ascontiguousarray(x.reshape(B * S, E))
    t_core = B * S // N_CORES
    assert t_core == T

    f64 = np.float64
    Wq64, Wk64 = np.asarray(Wq, f64), np.asarray(Wk, f64)
    Wv64, Wo64 = np.asarray(Wv, f64), np.asarray(Wo, f64)
    bq64, bk64 = np.asarray(bq, f64), np.asarray(bk, f64)
    bv64, bo64 = np.asarray(bv, f64), np.asarray(bo, f64)

    M = np.ascontiguousarray((Wq64.T @ Wk64).astype(np.float32))
    N = np.ascontiguousarray((Wv64.T @ Wo64.T).astype(np.float32))
    ck = (x_flat.astype(f64) @ (Wk64.T @ bq64)) * SCALE  # [T_total]
    orow = (bv64 @ Wo64.T + bo64).astype(np.float32)  # [E]

    common = {"m": M, "n": N}
    in_maps = [
        {
            "xt": np.ascontiguousarray(x_flat[i * t_core : (i + 1) * t_core].T),
            # ck columns: [128, 2*nw], one column per 128-token chunk
            "ckc": np.ascontiguousarray(
                ck[i * t_core : (i + 1) * t_core]
                .astype(np.float32)
                .reshape(2 * NW, 128)
                .T
            ),
            **common,
        }
        for i in range(N_CORES)
    ]
    return in_maps, orow, x_flat, (B, S)


def kernel(x, Wq, bq, Wk, bk, Wv, bv, Wo, bo):
    in_maps, orow, x_flat, (B, S) = prepare(x, Wq, bq, Wk, bk, Wv, bv, Wo, bo)
    nc = _get_nc()
    res = run_bass_kernel_spmd(nc, in_maps, core_ids=list(range(N_CORES)))
    out = np.concatenate([res.results[i]["o"] for i in range(N_CORES)], axis=0)
    out += orow[None, :]
    out += x_flat
    return out.reshape(B, S, E).astype(np.float32)
